# revision 1
# baseline (speedup 1.0000x reference)
"""CoAtten2 Trainium2 kernel: 8-way tensor-parallel over one TRN2 chip.

Reference computation (C=1024, H=W=64, HW=4096):
    q   = (Wq @ Xm + bq)  viewed [1024, 2048] then transposed
    kf  = (Wk1 @ Xf + bk1) viewed [1024, 2048]
    kl  = (Wk2 @ Xl + bk2) viewed [1024, 2048]
    att = softmax(kf @ q) + softmax(kl @ q)          # [1024, 1024]
    out = gamma * (att @ (Wv @ Xm + bv)) + (Xf + Xl)/2

Decomposition (per core d of 8; group t = d//4, rank r = d%4):
  - Channel indices are permuted (I' = 512t + o <-> i = 2o + t) so the
    torch-style reshape becomes contiguous; the permutation is folded into the
    host-side Wv/bv prep and the output DMA access pattern; gamma into Wv/bv.
  - logits_PERM splits into parity quadrants Q(t, t') whose kf operand needs
    spatial columns [2048t, 2048t+2048) and whose q operand needs spatial
    columns [2048t', ...). Core d owns spatial slice S_d = [512d, 512(d+1));
    it computes the partial contraction over S_d of Q(t, t'=0 and 1) for both
    attention branches from local projections.
  - All matmul operands are fp16 (1 cycle/row on the PE, vs ~2-4 for f32r);
    PSUM accumulation stays fp32.  CPU simulation of this exact quantization
    scheme gives rel_err 6.8e-3 (gate 2e-2).
  - One 4-way fp16 ReduceScatter over the f/l-interleaved [1024, 1024]
    partials deals each core its (f, l) 128-row logits pair; a tiny dep-free
    RS on the same communicator absorbs the first-op penalty, and a small
    AllGather queued behind the RS fills the ncfw gap so the real AllGather
    fast-follows warm.  V projection + residual prep overlap the RS window.
  - softmax is a free-dim reduction; summed attention is AllGathered in fp16;
    the output phase reads att^T back via DMA-transpose and accumulates
    att^T_chunk @ V_chunk per output tile, plus residual.
"""

import sys

sys.path.insert(0, "/opt/trn_rl_repo")

import numpy as np

import concourse.bacc as bacc
import concourse.mybir as mybir
from concourse import tile
from concourse.bass_utils import run_bass_kernel_spmd

F32 = mybir.dt.float32
F16 = mybir.dt.float16

C = 1024
HW = 4096
S = 512          # spatial columns per core
CH = 512         # C // 2 (projection output channels)
NCORES = 8

_CACHE: dict = {}


def _build():
    nc = bacc.Bacc("TRN2", target_bir_lowering=False, debug=False, num_devices=NCORES)

    # per-core external inputs (all fp16 data path; fp32 bias rows for adds)
    xm = nc.declare_dram_parameter("xm", [C, S], F16, isOutput=False)
    xf = nc.declare_dram_parameter("xf", [C, S], F16, isOutput=False)
    xl = nc.declare_dram_parameter("xl", [C, S], F16, isOutput=False)
    xq0 = nc.declare_dram_parameter("xq0", [C, S], F16, isOutput=False)  # Xm block d%4
    xq1 = nc.declare_dram_parameter("xq1", [C, S], F16, isOutput=False)  # Xm block 4+d%4
    wq = nc.declare_dram_parameter("wq", [C, CH], F16, isOutput=False)   # Wq.T
    wk1 = nc.declare_dram_parameter("wk1", [C, CH], F16, isOutput=False)
    wk2 = nc.declare_dram_parameter("wk2", [C, CH], F16, isOutput=False)
    wv = nc.declare_dram_parameter("wv", [C, C], F16, isOutput=False)    # (g*Wv)[permJ].T
    bqr = nc.declare_dram_parameter("bqr", [128, CH], F32, isOutput=False)
    bk1r = nc.declare_dram_parameter("bk1r", [128, CH], F32, isOutput=False)
    bk2r = nc.declare_dram_parameter("bk2r", [128, CH], F32, isOutput=False)
    bvp = nc.declare_dram_parameter("bvp", [128, 8], F32, isOutput=False)
    rsd = nc.declare_dram_parameter("rsd", [C, S], F16, isOutput=False)  # resid, perm rows
    out_ext = nc.declare_dram_parameter("out", [C, S], F16, isOutput=True)

    # internal DRAM
    rs_in = nc.dram_tensor("rs_in", [2 * CH, C], F16)   # f/l interleaved
    rs_out = nc.dram_tensor("rs_out", [256, C], F16)
    att_in = nc.dram_tensor("att_in", [128, C], F16)
    att_out = nc.dram_tensor("att_out", [C, C], F16, addr_space="Shared")
    dmy_in = nc.dram_tensor("dmy_in", [128, 16], F16)
    dmy_out = nc.dram_tensor("dmy_out", [32, 16], F16)
    dmy8_out = nc.dram_tensor("dmy8_out", [128, C], F16, addr_space="Shared")

    groups8 = [list(range(NCORES))]
    groups4 = [[0, 1, 2, 3], [4, 5, 6, 7]]

    with tile.TileContext(nc) as tc:
        with (
            tc.tile_pool(name="pw", bufs=1) as pw,
            tc.tile_pool(name="psg", bufs=3) as psg,
            tc.tile_pool(name="psc", bufs=2) as psc,
            tc.tile_pool(name="pps", bufs=4, space="PSUM") as pps,
            tc.tile_pool(name="plog", bufs=2, space="PSUM") as plog,
        ):
            # dep-free tiny RS on the same communicator as the real RSs: it
            # absorbs the first-op penalty during the rendezvous barrier and
            # lets RS_f fast-follow with a pending trigger
            nc.gpsimd.collective_compute(
                "ReduceScatter",
                mybir.AluOpType.add,
                ins=[dmy_in[:]],
                outs=[dmy_out[:]],
                replica_groups=groups4,
            )

            # ---- merged input loads: one DMA per tensor, spread over both
            # HWDGE rings (sync + scalar) ------------------------------------
            # dram [1024, W] -> sbuf [128, 8*W]; chunk c lives at cols [W*c,)
            def load_x(dram, width, tag, eng, split=1):
                # split>1: issue per-piece DMAs so the first chunks land (and
                # unblock their matmuls) before the whole tensor arrives
                t = pw.tile([128, 8 * width], F16, tag=tag)
                cs = 8 // split
                for i in range(split):
                    eng.dma_start(
                        t[:, cs * width * i:cs * width * (i + 1)].rearrange(
                            "p (c w) -> p c w", c=cs
                        ),
                        dram[128 * cs * i:128 * cs * (i + 1), :].rearrange(
                            "(c p) w -> p c w", p=128
                        ),
                    )
                return t

            def bias_tile(dram, tag, eng):
                t = pw.tile([128, CH], F32, tag=tag)
                eng.dma_start(t[:], dram[:, :])
                return t

            xf_t = load_x(xf, S, "xf", nc.sync, split=2)
            wk1_t = load_x(wk1, CH, "wk1", nc.scalar, split=2)
            bk1_t = bias_tile(bk1r, "bk1", nc.scalar)
            wq_t = load_x(wq, CH, "wq", nc.scalar)
            bq_t = bias_tile(bqr, "bq", nc.scalar)
            xq0_t = load_x(xq0, S, "xq0", nc.sync)
            xq1_t = load_x(xq1, S, "xq1", nc.sync)
            xl_t = load_x(xl, S, "xl", nc.sync)
            wk2_t = load_x(wk2, CH, "wk2", nc.scalar)
            bk2_t = bias_tile(bk2r, "bk2", nc.scalar)
            xm_t = load_x(xm, S, "xm", nc.sync)
            wv_t = load_x(wv, C, "wv", nc.scalar)
            bv_t = pw.tile([128, 8], F32, tag="bv")
            nc.scalar.dma_start(bv_t[:], bvp[:, :])

            # ---- local transposed projections -------------------------------
            # proj(X, WT, b)[s, o] = sum_c X[c, s] WT[c, o] + b[o]  -> [512, 512]
            # stays in SBUF as 4 [128, 512] fp16 tiles (s on partitions).
            def proj(x_t, w_t, b_t, otag):
                outs = []
                for ssub in range(4):
                    ps = pps.tile([128, CH], F32, tag="mm")
                    for c in range(8):
                        nc.tensor.matmul(
                            ps[:],
                            x_t[:, S * c + 128 * ssub:S * c + 128 * (ssub + 1)],
                            w_t[:, CH * c:CH * (c + 1)],
                            start=(c == 0),
                            stop=(c == 7),
                        )
                    o = pw.tile([128, CH], F16, tag=f"{otag}{ssub}")
                    nc.vector.tensor_add(o[:], ps[:], b_t[:])
                    outs.append(o)
                return outs

            def partials(ck, cq, branch):
                # o-tile m: partial[128 o, 512 t'-block] over local s; f/l
                # interleave per 256-row chunk so the single RS deals each
                # rank its own (f, l) 128-row pair
                for m in range(4):
                    psl = plog.tile([128, C], F32, tag="lg")
                    for tp in range(2):
                        for k in range(4):
                            nc.tensor.matmul(
                                psl[:, CH * tp:CH * (tp + 1)],
                                ck[k][:, 128 * m:128 * (m + 1)],
                                cq[tp][k][:],
                                start=(k == 0),
                                stop=(k == 3),
                            )
                    stg = psg.tile([128, C], F16, tag="stg")
                    nc.vector.tensor_copy(stg[:], psl[:])
                    row = 256 * m + 128 * branch
                    nc.sync.dma_start(rs_in[row:row + 128, :], stg[:])

            # f-branch chain first: RS_f's trigger is ready at the barrier
            # floor; RS_l's trigger pends during RS_f and fast-follows it
            ckf = proj(xf_t, wk1_t, bk1_t, "ckf")
            cq0 = proj(xq0_t, wq_t, bq_t, "cq0")
            cq1 = proj(xq1_t, wq_t, bq_t, "cq1")
            cq = [cq0, cq1]
            partials(ckf, cq, 0)
            ckl = proj(xl_t, wk2_t, bk2_t, "ckl")
            partials(ckl, cq, 1)
            nc.gpsimd.collective_compute(
                "ReduceScatter",
                mybir.AluOpType.add,
                ins=[rs_in[:]],
                outs=[rs_out[:]],
                replica_groups=groups4,
            )
            # small AllGather reading the tail of rs_in_l: its trigger fires
            # with RS_l's (same staging deps), pends behind RS_l, fills the
            # ncfw idle gap while softmax runs, and the real AllGather
            # fast-follows on the warmed groups8 communicator
            nc.gpsimd.collective_compute(
                "AllGather",
                mybir.AluOpType.bypass,
                ins=[rs_in[1008:1024, :]],
                outs=[dmy8_out[:]],
                replica_groups=groups8,
            )

            # ---- V projection (local, overlaps the RS/AG window) ------------
            # V[J', hw_d] fp16, bias per J' partition
            v_sb = []
            for j in range(8):
                ps = pps.tile([128, S], F32, tag="mm")
                for c in range(8):
                    nc.tensor.matmul(
                        ps[:],
                        wv_t[:, C * c + 128 * j:C * c + 128 * (j + 1)],
                        xm_t[:, S * c:S * (c + 1)],
                        start=(c == 0),
                        stop=(c == 7),
                    )
                v = pw.tile([128, S], F16, tag=f"v{j}")
                nc.vector.tensor_scalar_add(v[:], ps[:], bv_t[:, j:j + 1])
                v_sb.append(v)

            # ---- residual: host-precomputed 0.5*(xf+xl), permuted rows ------
            # tile e at cols [512e, 512e+512) = permuted rows 128e..128e+128
            rs_t = load_x(rsd, S, "rsd", nc.scalar)

            # ---- softmax on the dealt (f, l) 128-row blocks -----------------
            lgs = []
            for ci in (0, 1):
                lg = pw.tile([128, C], F16, tag=f"lg{ci}")
                (nc.sync if ci == 0 else nc.scalar).dma_start(
                    lg[:], rs_out[128 * ci:128 * (ci + 1), :]
                )
                lgs.append(lg)
            mxs, sms, att_parts = [], [], []
            for ci in (0, 1):
                mxn = psc.tile([128, 1], F32, tag=f"mx{ci}")
                nc.vector.reduce_max(
                    mxn[:], lgs[ci][:], axis=mybir.AxisListType.X, negate=True
                )
                mxs.append(mxn)
            for ci in (0, 1):
                sm = psc.tile([128, 1], F32, tag=f"sm{ci}")
                at = pw.tile([128, C], F16, tag=f"at{ci}")
                nc.scalar.activation(
                    at[:],
                    lgs[ci][:],
                    mybir.ActivationFunctionType.Exp,
                    bias=mxs[ci][:, 0:1],
                    accum_out=sm[:, 0:1],
                )
                sms.append(sm)
                att_parts.append(at)
            for ci in (0, 1):
                rcp = psc.tile([128, 1], F32, tag=f"rc{ci}")
                nc.vector.reciprocal(rcp[:], sms[ci][:])
                nc.vector.tensor_scalar_mul(
                    att_parts[ci][:], att_parts[ci][:], rcp[:, 0:1]
                )
            att_sum = pw.tile([128, C], F16, tag="atsum")
            nc.vector.tensor_add(att_sum[:], att_parts[0][:], att_parts[1][:])
            nc.sync.dma_start(att_in[:, 0:CH], att_sum[:, 0:CH])
            nc.scalar.dma_start(att_in[:, CH:C], att_sum[:, CH:C])
            nc.gpsimd.collective_compute(
                "AllGather",
                mybir.AluOpType.bypass,
                ins=[att_in[:]],
                outs=[att_out[:]],
                replica_groups=groups8,
            )

            # ---- out[:, hw_d] = att @ V_d + R -------------------------------
            # k-outer accumulation into 8 live PSUM accumulators (4 pps banks
            # + 2 plog tiles split in half): matmuls for chunk k start as soon
            # as its DMA-transposed read lands (xbar path: sync ring only)
            ps_out = []
            for i in range(2):
                big = plog.tile([128, C], F32, tag="lg")
                ps_out += [big[:, 0:S], big[:, S:C]]
            for i in range(4):
                small = pps.tile([128, S], F32, tag="mm")
                ps_out.append(small[:])
            att_t = []
            for k in range(8):
                at = pw.tile([128, C], F16, tag=f"attt{k}")
                nc.sync.dma_start(
                    at[:], att_out[:, 128 * k:128 * (k + 1)], transpose=True
                )
                att_t.append(at)
            # triangular wavefront: accumulator e starts at wave e, consuming
            # chunks in arrival order, so stops stagger and each residual-add
            # + output DMA pipelines into the remaining matmul stream.  rows
            # stay in permuted I' order; the host un-permutes for free
            for t in range(15):
                for e in range(max(0, t - 7), min(8, t + 1)):
                    c = t - e
                    nc.tensor.matmul(
                        ps_out[e],
                        att_t[c][:, 128 * e:128 * (e + 1)],
                        v_sb[c][:],
                        start=(c == 0),
                        stop=(c == 7),
                    )
                    if c == 7:
                        ost = psg.tile([128, S], F16, tag="ost")
                        nc.vector.tensor_add(
                            ost[:], ps_out[e], rs_t[:, S * e:S * (e + 1)]
                        )
                        (nc.sync if e % 2 == 0 else nc.scalar).dma_start(
                            out_ext[128 * e:128 * (e + 1), :], ost[:]
                        )

    nc.compile()
    return nc


def _prep_inputs(x_f, x_m, x_l, Wq, bq, Wk1, bk1, Wk2, bk2, Wv, bv, gamma):
    Xf = np.ascontiguousarray(x_f.reshape(C, HW), dtype=np.float16)
    Xm = np.ascontiguousarray(x_m.reshape(C, HW), dtype=np.float16)
    Xl = np.ascontiguousarray(x_l.reshape(C, HW), dtype=np.float16)
    g = np.float64(np.asarray(gamma).reshape(-1)[0])

    permJ = 2 * (np.arange(C) % 512) + np.arange(C) // 512  # J' -> global j
    wv_full = np.ascontiguousarray(
        (g * Wv.astype(np.float64))[permJ, :].T, dtype=np.float16
    )
    bv_perm = (g * bv.astype(np.float64))[permJ].astype(np.float32)

    wq_full = np.ascontiguousarray(Wq.T, dtype=np.float16)
    wk1_full = np.ascontiguousarray(Wk1.T, dtype=np.float16)
    wk2_full = np.ascontiguousarray(Wk2.T, dtype=np.float16)
    bqr = np.ascontiguousarray(np.broadcast_to(bq, (128, CH)), dtype=np.float32)
    bk1r = np.ascontiguousarray(np.broadcast_to(bk1, (128, CH)), dtype=np.float32)
    bk2r = np.ascontiguousarray(np.broadcast_to(bk2, (128, CH)), dtype=np.float32)
    bvp = np.ascontiguousarray(bv_perm.reshape(8, 128).T)
    Rp = (0.5 * (x_f.reshape(C, HW).astype(np.float64)
                 + x_l.reshape(C, HW).astype(np.float64)))[permJ].astype(np.float16)

    in_maps = []
    for d in range(NCORES):
        sl = slice(S * d, S * (d + 1))
        s0 = slice(S * (d % 4), S * (d % 4 + 1))
        s1 = slice(S * (4 + d % 4), S * (4 + d % 4 + 1))
        in_maps.append({
            "xm": np.ascontiguousarray(Xm[:, sl]),
            "xf": np.ascontiguousarray(Xf[:, sl]),
            "xl": np.ascontiguousarray(Xl[:, sl]),
            "xq0": np.ascontiguousarray(Xm[:, s0]),
            "xq1": np.ascontiguousarray(Xm[:, s1]),
            "wq": wq_full,
            "wk1": wk1_full,
            "wk2": wk2_full,
            "wv": wv_full,
            "bqr": bqr,
            "bk1r": bk1r,
            "bk2r": bk2r,
            "bvp": bvp,
            "rsd": np.ascontiguousarray(Rp[:, sl]),
        })
    return in_maps


def _run(inputs: dict, trace: bool = False, **kw):
    if "nc" not in _CACHE:
        _CACHE["nc"] = _build()
    nc = _CACHE["nc"]
    in_maps = _prep_inputs(**inputs)
    res = run_bass_kernel_spmd(nc, in_maps, list(range(NCORES)), trace=trace, **kw)
    permJ = 2 * (np.arange(C) % 512) + np.arange(C) // 512
    out = np.empty((C, HW), np.float32)
    for d in range(NCORES):
        out[permJ, S * d:S * (d + 1)] = res.results[d]["out"].astype(np.float32)
    return out.reshape(1, C, 64, 64), res


def kernel(**inputs) -> np.ndarray:
    inputs = {k: np.asarray(v) for k, v in inputs.items()}
    try:
        out, _ = _run(inputs)
    except Exception:
        out, _ = _run(inputs)  # retry once on transient device/runtime errors
    return out



# revision 47
# speedup vs baseline: 1.0604x; 1.0604x over previous
"""CoAtten2 Trainium2 kernel: 8-way tensor-parallel over one TRN2 chip.

Reference computation (C=1024, H=W=64, HW=4096):
    q   = (Wq @ Xm + bq)  viewed [1024, 2048] then transposed
    kf  = (Wk1 @ Xf + bk1) viewed [1024, 2048]
    kl  = (Wk2 @ Xl + bk2) viewed [1024, 2048]
    att = softmax(kf @ q) + softmax(kl @ q)          # [1024, 1024]
    out = gamma * (att @ (Wv @ Xm + bv)) + (Xf + Xl)/2

Decomposition (per core d of 8; group t = d//4, rank r = d%4):
  - Channel indices are permuted (I' = 512t + o <-> i = 2o + t) so the
    torch-style reshape becomes contiguous; the permutation is folded into the
    host-side Wv/bv prep and the output DMA access pattern; gamma into Wv/bv.
  - logits_PERM splits into parity quadrants Q(t, t') whose kf operand needs
    spatial columns [2048t, 2048t+2048) and whose q operand needs spatial
    columns [2048t', ...). Core d owns spatial slice S_d = [512d, 512(d+1));
    it computes the partial contraction over S_d of Q(t, t'=0 and 1) for both
    attention branches from local projections.
  - All matmul operands are fp16 (1 cycle/row on the PE, vs ~2-4 for f32r);
    PSUM accumulation stays fp32.  CPU simulation of this exact quantization
    scheme gives rel_err 6.8e-3 (gate 2e-2).
  - One 4-way fp16 ReduceScatter over the f/l-interleaved [1024, 1024]
    partials deals each core its (f, l) 128-row logits pair; a tiny dep-free
    RS on the same communicator absorbs the first-op penalty, and a small
    AllGather queued behind the RS fills the ncfw gap so the real AllGather
    fast-follows warm.  V projection + residual prep overlap the RS window.
  - softmax is a free-dim reduction; summed attention is AllGathered in fp16;
    the output phase reads att^T back via DMA-transpose and accumulates
    att^T_chunk @ V_chunk per output tile, plus residual.
"""

import sys

sys.path.insert(0, "/opt/trn_rl_repo")

import numpy as np

import concourse.bacc as bacc
import concourse.mybir as mybir
from concourse import tile
from concourse.bass_utils import run_bass_kernel_spmd

F32 = mybir.dt.float32
F16 = mybir.dt.float16

C = 1024
HW = 4096
S = 512          # spatial columns per core
CH = 512         # C // 2 (projection output channels)
NCORES = 8

_CACHE: dict = {}


def _build():
    nc = bacc.Bacc("TRN2", target_bir_lowering=False, debug=False, num_devices=NCORES)

    # per-core external inputs (all fp16 data path; fp32 bias rows for adds)
    xm = nc.declare_dram_parameter("xm", [C, S], F16, isOutput=False)
    xf = nc.declare_dram_parameter("xf", [C, S], F16, isOutput=False)
    xl = nc.declare_dram_parameter("xl", [C, S], F16, isOutput=False)
    xq0 = nc.declare_dram_parameter("xq0", [C, S], F16, isOutput=False)  # Xm block d%4
    xq1 = nc.declare_dram_parameter("xq1", [C, S], F16, isOutput=False)  # Xm block 4+d%4
    wq = nc.declare_dram_parameter("wq", [C, CH], F16, isOutput=False)   # Wq.T
    wk1 = nc.declare_dram_parameter("wk1", [C, CH], F16, isOutput=False)
    wk2 = nc.declare_dram_parameter("wk2", [C, CH], F16, isOutput=False)
    wv = nc.declare_dram_parameter("wv", [C, C], F16, isOutput=False)    # (g*Wv)[permJ].T
    bqr = nc.declare_dram_parameter("bqr", [128, CH], F32, isOutput=False)
    bk1r = nc.declare_dram_parameter("bk1r", [128, CH], F32, isOutput=False)
    bk2r = nc.declare_dram_parameter("bk2r", [128, CH], F32, isOutput=False)
    bvp = nc.declare_dram_parameter("bvp", [128, 8], F32, isOutput=False)
    rsd = nc.declare_dram_parameter("rsd", [C, S], F16, isOutput=False)  # resid, perm rows
    out_ext = nc.declare_dram_parameter("out", [C, S], F16, isOutput=True)

    # internal DRAM
    rs_in = nc.dram_tensor("rs_in", [2 * CH, C], F16)   # f/l interleaved
    rs_out = nc.dram_tensor("rs_out", [256, C], F16)
    att_in = nc.dram_tensor("att_in", [128, C], F16)
    att_out = nc.dram_tensor("att_out", [C, C], F16, addr_space="Shared")
    dmy_in = nc.dram_tensor("dmy_in", [128, 16], F16)
    dmy_out = nc.dram_tensor("dmy_out", [32, 16], F16)
    dmy8_out = nc.dram_tensor("dmy8_out", [128, C], F16, addr_space="Shared")

    groups8 = [list(range(NCORES))]
    groups4 = [[0, 1, 2, 3], [4, 5, 6, 7]]

    with tile.TileContext(nc) as tc:
        with (
            tc.tile_pool(name="pw", bufs=1) as pw,
            tc.tile_pool(name="psg", bufs=3) as psg,
            tc.tile_pool(name="psc", bufs=2) as psc,
            tc.tile_pool(name="pps", bufs=4, space="PSUM") as pps,
            tc.tile_pool(name="plog", bufs=2, space="PSUM") as plog,
        ):
            # dep-free tiny RS on the same communicator as the real RSs: it
            # absorbs the first-op penalty during the rendezvous barrier and
            # lets RS_f fast-follow with a pending trigger
            nc.gpsimd.collective_compute(
                "ReduceScatter",
                mybir.AluOpType.add,
                ins=[dmy_in[:]],
                outs=[dmy_out[:]],
                replica_groups=groups4,
            )

            # ---- merged input loads: one DMA per tensor, spread over both
            # HWDGE rings (sync + scalar) ------------------------------------
            # dram [1024, W] -> sbuf [128, 8*W]; chunk c lives at cols [W*c,)
            def load_x(dram, width, tag, eng, split=1):
                # split>1: issue per-piece DMAs so the first chunks land (and
                # unblock their matmuls) before the whole tensor arrives
                t = pw.tile([128, 8 * width], F16, tag=tag)
                cs = 8 // split
                for i in range(split):
                    eng.dma_start(
                        t[:, cs * width * i:cs * width * (i + 1)].rearrange(
                            "p (c w) -> p c w", c=cs
                        ),
                        dram[128 * cs * i:128 * cs * (i + 1), :].rearrange(
                            "(c p) w -> p c w", p=128
                        ),
                    )
                return t

            def bias_tile(dram, tag, eng):
                t = pw.tile([128, CH], F32, tag=tag)
                eng.dma_start(t[:], dram[:, :])
                return t

            xf_t = load_x(xf, S, "xf", nc.sync, split=2)
            wk1_t = load_x(wk1, CH, "wk1", nc.scalar, split=2)
            bk1_t = bias_tile(bk1r, "bk1", nc.scalar)
            wq_t = load_x(wq, CH, "wq", nc.scalar)
            bq_t = bias_tile(bqr, "bq", nc.scalar)
            xq0_t = load_x(xq0, S, "xq0", nc.sync)
            xq1_t = load_x(xq1, S, "xq1", nc.sync)
            xl_t = load_x(xl, S, "xl", nc.sync)
            wk2_t = load_x(wk2, CH, "wk2", nc.scalar)
            bk2_t = bias_tile(bk2r, "bk2", nc.scalar)
            xm_t = load_x(xm, S, "xm", nc.sync)
            wv_t = load_x(wv, C, "wv", nc.scalar)
            bv_t = pw.tile([128, 8], F32, tag="bv")
            nc.scalar.dma_start(bv_t[:], bvp[:, :])

            # ---- local transposed projections -------------------------------
            # proj(X, WT, b)[s, o] = sum_c X[c, s] WT[c, o] + b[o]  -> [512, 512]
            # stays in SBUF as 4 [128, 512] fp16 tiles (s on partitions).
            def proj(x_t, w_t, b_t, otag):
                outs = []
                for ssub in range(4):
                    ps = pps.tile([128, CH], F32, tag="mm")
                    for c in range(8):
                        nc.tensor.matmul(
                            ps[:],
                            x_t[:, S * c + 128 * ssub:S * c + 128 * (ssub + 1)],
                            w_t[:, CH * c:CH * (c + 1)],
                            start=(c == 0),
                            stop=(c == 7),
                        )
                    o = pw.tile([128, CH], F16, tag=f"{otag}{ssub}")
                    nc.vector.tensor_add(o[:], ps[:], b_t[:])
                    outs.append(o)
                return outs

            def partials(ck, cq, branch):
                # o-tile m: partial[128 o, 512 t'-block] over local s; f/l
                # interleave per 256-row chunk so the single RS deals each
                # rank its own (f, l) 128-row pair
                for m in range(4):
                    psl = plog.tile([128, C], F32, tag="lg")
                    for tp in range(2):
                        for k in range(4):
                            nc.tensor.matmul(
                                psl[:, CH * tp:CH * (tp + 1)],
                                ck[k][:, 128 * m:128 * (m + 1)],
                                cq[tp][k][:],
                                start=(k == 0),
                                stop=(k == 3),
                            )
                    stg = psg.tile([128, C], F16, tag="stg")
                    nc.vector.tensor_copy(stg[:], psl[:])
                    row = 256 * m + 128 * branch
                    nc.sync.dma_start(rs_in[row:row + 128, :], stg[:])

            # f-branch chain first: RS_f's trigger is ready at the barrier
            # floor; RS_l's trigger pends during RS_f and fast-follows it
            ckf = proj(xf_t, wk1_t, bk1_t, "ckf")
            cq0 = proj(xq0_t, wq_t, bq_t, "cq0")
            cq1 = proj(xq1_t, wq_t, bq_t, "cq1")
            cq = [cq0, cq1]
            partials(ckf, cq, 0)
            ckl = proj(xl_t, wk2_t, bk2_t, "ckl")
            partials(ckl, cq, 1)
            nc.gpsimd.collective_compute(
                "ReduceScatter",
                mybir.AluOpType.add,
                ins=[rs_in[:]],
                outs=[rs_out[:]],
                replica_groups=groups4,
            )
            # small AllGather reading the tail of rs_in_l: its trigger fires
            # with RS_l's (same staging deps), pends behind RS_l, fills the
            # ncfw idle gap while softmax runs, and the real AllGather
            # fast-follows on the warmed groups8 communicator
            nc.gpsimd.collective_compute(
                "AllGather",
                mybir.AluOpType.bypass,
                ins=[rs_in[1008:1024, :]],
                outs=[dmy8_out[:]],
                replica_groups=groups8,
            )

            # ---- V projection (local, overlaps the RS/AG window) ------------
            # V[J', hw_d] fp16, bias per J' partition
            v_sb = []
            for j in range(8):
                ps = pps.tile([128, S], F32, tag="mm")
                for c in range(8):
                    nc.tensor.matmul(
                        ps[:],
                        wv_t[:, C * c + 128 * j:C * c + 128 * (j + 1)],
                        xm_t[:, S * c:S * (c + 1)],
                        start=(c == 0),
                        stop=(c == 7),
                    )
                v = pw.tile([128, S], F16, tag=f"v{j}")
                nc.vector.tensor_scalar_add(v[:], ps[:], bv_t[:, j:j + 1])
                v_sb.append(v)

            # ---- residual: host-precomputed 0.5*(xf+xl), permuted rows ------
            # tile e at cols [512e, 512e+512) = permuted rows 128e..128e+128
            rs_t = load_x(rsd, S, "rsd", nc.scalar)

            # ---- softmax on the dealt (f, l) 128-row blocks -----------------
            lgs = []
            for ci in (0, 1):
                lg = pw.tile([128, C], F16, tag=f"lg{ci}")
                (nc.sync if ci == 0 else nc.scalar).dma_start(
                    lg[:], rs_out[128 * ci:128 * (ci + 1), :]
                )
                lgs.append(lg)
            mxs, sms, att_parts = [], [], []
            for ci in (0, 1):
                mxn = psc.tile([128, 1], F32, tag=f"mx{ci}")
                nc.vector.reduce_max(
                    mxn[:], lgs[ci][:], axis=mybir.AxisListType.X, negate=True
                )
                mxs.append(mxn)
            for ci in (0, 1):
                sm = psc.tile([128, 1], F32, tag=f"sm{ci}")
                at = pw.tile([128, C], F16, tag=f"at{ci}")
                nc.scalar.activation(
                    at[:],
                    lgs[ci][:],
                    mybir.ActivationFunctionType.Exp,
                    bias=mxs[ci][:, 0:1],
                    accum_out=sm[:, 0:1],
                )
                sms.append(sm)
                att_parts.append(at)
            for ci in (0, 1):
                rcp = psc.tile([128, 1], F32, tag=f"rc{ci}")
                nc.vector.reciprocal(rcp[:], sms[ci][:])
                nc.vector.tensor_scalar_mul(
                    att_parts[ci][:], att_parts[ci][:], rcp[:, 0:1]
                )
            att_sum = pw.tile([128, C], F16, tag="atsum")
            nc.vector.tensor_add(att_sum[:], att_parts[0][:], att_parts[1][:])
            nc.sync.dma_start(att_in[:, 0:CH], att_sum[:, 0:CH])
            nc.scalar.dma_start(att_in[:, CH:C], att_sum[:, CH:C])
            nc.gpsimd.collective_compute(
                "AllGather",
                mybir.AluOpType.bypass,
                ins=[att_in[:]],
                outs=[att_out[:]],
                replica_groups=groups8,
            )

            # ---- out[:, hw_d] = att @ V_d + R -------------------------------
            # k-outer accumulation into 8 live PSUM accumulators (4 pps banks
            # + 2 plog tiles split in half): matmuls for chunk k start as soon
            # as its DMA-transposed read lands (xbar path: sync ring only)
            ps_out = []
            for i in range(2):
                big = plog.tile([128, C], F32, tag="lg")
                ps_out += [big[:, 0:S], big[:, S:C]]
            for i in range(4):
                small = pps.tile([128, S], F32, tag="mm")
                ps_out.append(small[:])
            att_t = []
            for k in range(8):
                at = pw.tile([128, C], F16, tag=f"attt{k}")
                nc.sync.dma_start(
                    at[:], att_out[:, 128 * k:128 * (k + 1)], transpose=True
                )
                att_t.append(at)
            # triangular wavefront: accumulator e starts at wave e, consuming
            # chunks in arrival order, so stops stagger and each residual-add
            # + output DMA pipelines into the remaining matmul stream.  rows
            # stay in permuted I' order; the host un-permutes for free
            for t in range(15):
                for e in range(max(0, t - 7), min(8, t + 1)):
                    c = t - e
                    nc.tensor.matmul(
                        ps_out[e],
                        att_t[c][:, 128 * e:128 * (e + 1)],
                        v_sb[c][:],
                        start=(c == 0),
                        stop=(c == 7),
                    )
                    if c == 7:
                        ost = psg.tile([128, S], F16, tag="ost")
                        nc.vector.tensor_add(
                            ost[:], ps_out[e], rs_t[:, S * e:S * (e + 1)]
                        )
                        (nc.sync if e % 2 == 0 else nc.scalar).dma_start(
                            out_ext[128 * e:128 * (e + 1), :], ost[:]
                        )

    nc.compile()
    return nc


def _prep_inputs(x_f, x_m, x_l, Wq, bq, Wk1, bk1, Wk2, bk2, Wv, bv, gamma):
    Xf = np.ascontiguousarray(x_f.reshape(C, HW), dtype=np.float16)
    Xm = np.ascontiguousarray(x_m.reshape(C, HW), dtype=np.float16)
    Xl = np.ascontiguousarray(x_l.reshape(C, HW), dtype=np.float16)
    g = np.float64(np.asarray(gamma).reshape(-1)[0])

    permJ = 2 * (np.arange(C) % 512) + np.arange(C) // 512  # J' -> global j
    wv_full = np.ascontiguousarray(
        (g * Wv.astype(np.float64))[permJ, :].T, dtype=np.float16
    )
    bv_perm = (g * bv.astype(np.float64))[permJ].astype(np.float32)

    wq_full = np.ascontiguousarray(Wq.T, dtype=np.float16)
    wk1_full = np.ascontiguousarray(Wk1.T, dtype=np.float16)
    wk2_full = np.ascontiguousarray(Wk2.T, dtype=np.float16)
    bqr = np.ascontiguousarray(np.broadcast_to(bq, (128, CH)), dtype=np.float32)
    bk1r = np.ascontiguousarray(np.broadcast_to(bk1, (128, CH)), dtype=np.float32)
    bk2r = np.ascontiguousarray(np.broadcast_to(bk2, (128, CH)), dtype=np.float32)
    bvp = np.ascontiguousarray(bv_perm.reshape(8, 128).T)
    Rp = (0.5 * (x_f.reshape(C, HW).astype(np.float64)
                 + x_l.reshape(C, HW).astype(np.float64)))[permJ].astype(np.float16)

    in_maps = []
    for d in range(NCORES):
        sl = slice(S * d, S * (d + 1))
        s0 = slice(S * (d % 4), S * (d % 4 + 1))
        s1 = slice(S * (4 + d % 4), S * (4 + d % 4 + 1))
        in_maps.append({
            "xm": np.ascontiguousarray(Xm[:, sl]),
            "xf": np.ascontiguousarray(Xf[:, sl]),
            "xl": np.ascontiguousarray(Xl[:, sl]),
            "xq0": np.ascontiguousarray(Xm[:, s0]),
            "xq1": np.ascontiguousarray(Xm[:, s1]),
            "wq": wq_full,
            "wk1": wk1_full,
            "wk2": wk2_full,
            "wv": wv_full,
            "bqr": bqr,
            "bk1r": bk1r,
            "bk2r": bk2r,
            "bvp": bvp,
            "rsd": np.ascontiguousarray(Rp[:, sl]),
        })
    return in_maps


def _run(inputs: dict, trace: bool = False, **kw):
    if "nc" not in _CACHE:
        _CACHE["nc"] = _build()
    nc = _CACHE["nc"]
    in_maps = _prep_inputs(**inputs)
    res = run_bass_kernel_spmd(nc, in_maps, list(range(NCORES)), trace=trace, **kw)
    permJ = 2 * (np.arange(C) % 512) + np.arange(C) // 512
    out = np.empty((C, HW), np.float32)
    for d in range(NCORES):
        out[permJ, S * d:S * (d + 1)] = res.results[d]["out"].astype(np.float32)
    return out.reshape(1, C, 64, 64), res


def kernel(**inputs) -> np.ndarray:
    inputs = {k: np.asarray(v) for k, v in inputs.items()}
    try:
        out, _ = _run(inputs)
    except Exception:
        out, _ = _run(inputs)  # retry once on transient device/runtime errors
    return out

